# revision 10
# baseline (speedup 1.0000x reference)
"""Distributed Trainium2 Bass kernel for the 1x1-conv multi-head attention block.

Reference computation (per batch b of 4):
    qkv = w_qkv @ x            x: [256, 4096] (channels x spatial), w_qkv: [384, 256]
    q,k,v = split(qkv); per head h (4 heads, d=32): q *= d**-0.5
    sim = q^T k;  attn = softmax(sim, axis=k);  o = attn @ v^T
    y = w_out @ o + b_out      w_out: [256, 128]

Distribution: 8 cores = 4 batches x 2 query-halves. Each core computes k/v
projections for all 4096 positions, q for its 2048 query positions, full
attention for its query half, and the output projection for those columns.
The host concatenates the 8 disjoint output blocks; no collective needed.

The kernel is exp-throughput-bound (33.5M softmax exponentials per core, ACT
does 128 lanes/cycle), so the exp work is split between two engines:
  - ACT: native Exp activation (PSUM fp32 -> SBUF bf16).
  - DVE: a custom 8-stage DVE op (EXP_BITS, installed over the unused
    ADD_RANGE_WRAP table row) that emits bf16 BIT PATTERNS via int16
    convert: t = rho + Fm + c2*(Fm^2 + A), where rho/Fm are the 128-aligned
    round/remainder of the pre-scaled score (magic-constant rounding) and
    the quadratic corrects the mantissa field. Max value error ~0.5%.
Scores are computed pre-scaled by K1 = 128*log2(e) (folded into w_q) and
pre-biased by B = 16192 (a 33rd contraction row in the q/k layouts) so the
custom op's frac extraction is wrap-free. The ACT path compensates with its
free scale/bias to produce the same global factor.

Layouts: scores simT [k_pos (partitions), q_pos (free)]; softmax denominators
via a ones-column in the v stationary (PE partition reduction); attn@v
consumes exp'd scores as the bf16 moving operand. q/k live in a pitch-64
layout (head pair per tile: rows 0-32 and 64-96 incl. the aug row) so the
33-row sim matmuls 2-way-pack into PE row-tile positions {0, 64}.
"""

import sys

if "/opt/trn_rl_repo" not in sys.path:
    sys.path.insert(0, "/opt/trn_rl_repo")

import numpy as np
import ml_dtypes

import concourse.bass as bass
import concourse.mybir as mybir
import concourse.tile as tile
from concourse import bacc
from concourse import dve_ops
from concourse.dve_spec import Spec, Src0, C0, C1, C2, lower as dve_lower
from concourse.dve_uop import DveOpSpec

BF16 = mybir.dt.bfloat16
F32 = mybir.dt.float32
I16 = mybir.dt.int16

N_CORES = 8
HEADS = 4
DIM_HEAD = 32
SCALE = DIM_HEAD ** -0.5

# ---- exp constants (see probe_dve.py derivation) ------------------------
K1 = float(128.0 * np.log2(np.e))        # score pre-scale (folded into wq)
BIAS_B = 16192.0                         # score pre-bias (aug row), == 64 mod 128
CBIG = float(np.float32(1.5 * 2 ** 30))  # magic round-to-128 constant
C2_FIT = 0.0026719477027654648           # mantissa parabola curvature
A_FIT = 19838.392578125                  # parabola constant / c2
GAMMA = 0.999614                         # global factor of the DVE path
ACT_SCALE = float(1.0 / K1)
ACT_BIAS = float(np.log(GAMMA) - BIAS_B / K1)

# exp engine split: of every 64 (ktile, pair) units per q-tile, this many
# go to ACT (rest to the DVE custom op)
N_ACT_OF_64 = 35

DEFER_ACCS = True   # emit attn@v matmuls one k-tile late (engine continuity)


# ---- custom DVE op installation -----------------------------------------
# New table rows crash the runtime in this environment, so EXP_BITS is
# installed over an existing row (ADD_RANGE_WRAP, unused here). The per-NEFF
# DVE table is regenerated from dve_ops.OPS at compile time.

_EXP_OP = None


def _install_exp_op():
    global _EXP_OP
    name = "ADD_RANGE_WRAP"
    if _EXP_OP is not None and dve_ops.CUSTOM_DVE_SPECS.get(name) is _EXP_OP.spec:
        return _EXP_OP
    a = Src0 + C0
    rho = a - C0
    fm = Src0 - rho
    v = fm * fm
    u = v + C1
    psi = u * C2
    g = psi + fm
    body = g + rho

    def ref(in0, in1, s0, s1, imm2):
        z = in0.astype(np.float32)
        aa = (z + np.float32(s0)).astype(np.float32)
        rr = (aa - np.float32(s0)).astype(np.float32)
        ff = (z - rr).astype(np.float32)
        vv = (ff * ff).astype(np.float32)
        uu = (vv + np.float32(s1)).astype(np.float32)
        pp = (uu * np.float32(imm2)).astype(np.float32)
        gg = (pp + ff).astype(np.float32)
        return (gg + rr).astype(np.float32)

    op = dve_ops.DveOp(name, Spec(body=body, reference=ref),
                       subdim=False, uops_sha={})
    row = dve_ops._SUB_OPCODE_FOR_NAME[name]
    for ver in ("v3", "v4"):
        sp = DveOpSpec(name=name, opcode=row,
                       uops=dve_lower(op.spec, ver=ver), rd1_en=False)
        op.uops_sha[ver] = sp.sha(ver)
    idx = [i for i, o in enumerate(dve_ops.OPS) if o.name == name][0]
    dve_ops.OPS[idx] = op
    dve_ops.CUSTOM_DVE_SPECS[name] = op.spec
    _EXP_OP = op
    return op


class Cfg:
    def __init__(self, seq=4096, q_half=2048, q_tile=512, dim=256):
        self.seq = seq
        self.q_half = q_half
        self.q_tile = q_tile
        self.dim = dim
        self.hidden = HEADS * DIM_HEAD
        self.n_ktiles = seq // 128
        self.n_qtiles = q_half // q_tile
        self.n_stiles = seq // q_tile
        self.n_ctiles = dim // 128


FULL = Cfg()


def build_nc(cfg=FULL, n_cores=N_CORES):
    exp_op = _install_exp_op()
    nc = bacc.Bacc("TRN2", target_bir_lowering=False, debug=False,
                   num_devices=n_cores)

    x_d = nc.dram_tensor("x", [cfg.n_ctiles, 128, cfg.seq], BF16,
                         kind="ExternalInput")
    xq_d = nc.dram_tensor("xq", [cfg.n_ctiles, 128, cfg.q_half], BF16,
                          kind="ExternalInput")
    wq_d = nc.dram_tensor("wq", [cfg.n_ctiles, 128, 128], BF16,
                          kind="ExternalInput")
    wk_d = nc.dram_tensor("wk", [cfg.n_ctiles, 128, 128], BF16,
                          kind="ExternalInput")
    wv_d = nc.dram_tensor("wv", [cfg.n_ctiles, 128, 128], BF16,
                          kind="ExternalInput")
    wo_d = nc.dram_tensor("wo", [128, cfg.dim], BF16, kind="ExternalInput")
    bo_d = nc.dram_tensor("bo", [cfg.n_ctiles, 128, 1], F32,
                          kind="ExternalInput")
    y_d = nc.dram_tensor("out", [cfg.n_ctiles, 128, cfg.q_half], F32,
                         kind="ExternalOutput")

    with tile.TileContext(nc) as tc:
        _kernel_body(tc, cfg, exp_op, x_d, xq_d, wq_d, wk_d, wv_d, wo_d,
                     bo_d, y_d)
    nc.compile()
    return nc


def _kernel_body(tc, cfg, exp_op, x_d, xq_d, wq_d, wk_d, wv_d, wo_d, bo_d,
                 y_d):
    nc = tc.nc
    NK, NQ, QT = cfg.n_ktiles, cfg.n_qtiles, cfg.q_tile
    NC_, NS = cfg.n_ctiles, cfg.n_stiles

    from contextlib import ExitStack
    with ExitStack() as ctx:
        ep = ctx.enter_context

        consts = ep(tc.tile_pool(name="consts", bufs=1))
        persist = ep(tc.tile_pool(name="persist", bufs=1))

        x_sb = persist.tile([128, NC_, cfg.seq], BF16, tag="x")
        xq_sb = persist.tile([128, NC_, cfg.q_half], BF16, tag="xq")
        wq_sb = consts.tile([128, NC_, 128], BF16, tag="wq")
        wk_sb = consts.tile([128, NC_, 128], BF16, tag="wk")
        wv_sb = consts.tile([128, NC_, 128], BF16, tag="wv")
        wo_sb = consts.tile([128, cfg.dim], BF16, tag="wo")
        bo_sb = consts.tile([128, NC_], F32, tag="bo")

        # Prime the ACT exp table before the PE-dense phases (one-time
        # ~1.3us table load otherwise lands on the first real exp).
        prime = consts.tile([128, 8], F32, tag="prime")
        nc.vector.memset(prime[:], 0.0)
        nc.scalar.activation(prime[:], prime[:],
                             mybir.ActivationFunctionType.Exp)

        act_bias_sb = consts.tile([128, 1], F32, tag="act_bias")
        nc.vector.memset(act_bias_sb[:], ACT_BIAS)
        act_bias_nb_sb = consts.tile([128, 1], F32, tag="act_bias_nb")
        nc.vector.memset(act_bias_nb_sb[:], float(np.log(GAMMA)))

        for c in range(NC_):
            nc.sync.dma_start(xq_sb[:, c, :], xq_d[c])
            nc.sync.dma_start(wq_sb[:, c, :], wq_d[c])
        for c in range(NC_):
            nc.sync.dma_start(wk_sb[:, c, :], wk_d[c])
            nc.sync.dma_start(wv_sb[:, c, :], wv_d[c])
            nc.sync.dma_start(x_sb[:, c, :], x_d[c])
        for c in range(NC_):
            nc.sync.dma_start(bo_sb[:, c : c + 1], bo_d[c])
        nc.sync.dma_start(wo_sb[:], wo_d[:])

        # ---- projections ------------------------------------------------
        # Packed layouts (head h at rows 32h) feed the 4-way ACT-ktile
        # sims; pitch-64 aug layouts (head pair p: rows 0-32 / 64-96 incl.
        # the bias row) feed the 33-row DVE-ktile sims. The pitch-64
        # copies are SBUF->SBUF DMAs from the packed layout (DMA engines
        # are idle; engines can't cross partitions cheaply).
        qT32_sb = persist.tile([128, cfg.q_half], BF16, tag="qT32")
        kT32_sb = persist.tile([128, cfg.seq], BF16, tag="kT32")
        qT_sb = persist.tile([128, 2, cfg.q_half], BF16, tag="qT")
        kT_sb = persist.tile([128, 2, cfg.seq], BF16, tag="kT")
        v_sb = persist.tile([128, NK, HEADS, 33], BF16, tag="v")

        # aug rows (partition rows 32 / 96 of each pair slot)
        nc.vector.memset(qT_sb[32:33, :, :], BIAS_B)
        nc.vector.memset(qT_sb[96:97, :, :], BIAS_B)
        nc.vector.memset(kT_sb[32:33, :, :], 1.0)
        nc.vector.memset(kT_sb[96:97, :, :], 1.0)
        nc.vector.memset(v_sb[:, :, :, 32], 1.0)

        with tc.tile_pool(name="proj_ps", bufs=4, space="PSUM") as proj_ps, \
             tc.tile_pool(name="projv_ps", bufs=4, space="PSUM") as projv_ps:
            for s in range(cfg.q_half // QT):
                pt = proj_ps.tile([128, QT], F32, tag="proj")
                for c in range(NC_):
                    nc.tensor.matmul(pt[:], wq_sb[:, c, :],
                                     xq_sb[:, c, s * QT : (s + 1) * QT],
                                     start=(c == 0), stop=(c == NC_ - 1))
                nc.vector.tensor_copy(
                    qT32_sb[:, s * QT : (s + 1) * QT], pt[:])
            for h in range(HEADS):
                nc.sync.dma_start(
                    qT_sb[64 * (h % 2) : 64 * (h % 2) + 32, h // 2, :],
                    qT32_sb[32 * h : 32 * h + 32, :])
            for s in range(NS):
                pt = proj_ps.tile([128, QT], F32, tag="proj")
                for c in range(NC_):
                    nc.tensor.matmul(pt[:], wk_sb[:, c, :],
                                     x_sb[:, c, s * QT : (s + 1) * QT],
                                     start=(c == 0), stop=(c == NC_ - 1))
                eng = nc.scalar.copy if s % 2 == 0 else nc.vector.tensor_copy
                eng(kT32_sb[:, s * QT : (s + 1) * QT], pt[:])
            for h in range(HEADS):
                nc.sync.dma_start(
                    kT_sb[64 * (h % 2) : 64 * (h % 2) + 32, h // 2, :],
                    kT32_sb[32 * h : 32 * h + 32, :])
            # v projection, spatial-major: out [128 spatial, 128 hidden]
            for t in range(NK):
                pt = projv_ps.tile([128, 128], F32, tag="projv")
                for c in range(NC_):
                    nc.tensor.matmul(pt[:],
                                     x_sb[:, c, t * 128 : (t + 1) * 128],
                                     wv_sb[:, c, :],
                                     start=(c == 0), stop=(c == NC_ - 1))
                nc.scalar.copy(
                    v_sb[:, t, :, 0:32],
                    pt[:].rearrange("p (h d) -> p h d", h=HEADS))

        # ---- attention --------------------------------------------------
        attn_ctx = ExitStack()
        with attn_ctx:
            sim_ps = attn_ctx.enter_context(
                tc.tile_pool(name="sim_ps", bufs=2, space="PSUM"))
            acc_ps = attn_ctx.enter_context(
                tc.tile_pool(name="acc_ps", bufs=2, space="PSUM"))
            e_pool = attn_ctx.enter_context(
                tc.tile_pool(name="e_pool", bufs=3))
            norm_pool = attn_ctx.enter_context(
                tc.tile_pool(name="norm", bufs=3))

            outn_sb = persist.tile([128, cfg.q_half], BF16, tag="outn")

            self_attention(nc, cfg, tc, exp_op, sim_ps, acc_ps, e_pool,
                           norm_pool, qT_sb, kT_sb, qT32_sb, kT32_sb,
                           v_sb, outn_sb, act_bias_sb, act_bias_nb_sb)

        # ---- output projection ------------------------------------------
        with tc.tile_pool(name="y", bufs=8) as y_pool, \
             tc.tile_pool(name="y_ps", bufs=8, space="PSUM") as yp_ps:
            for c in range(NC_):
                for s in range(cfg.q_half // QT):
                    pt = yp_ps.tile([128, QT], F32, tag="yp")
                    nc.tensor.matmul(pt[:],
                                     wo_sb[:, c * 128 : (c + 1) * 128],
                                     outn_sb[:, s * QT : (s + 1) * QT],
                                     start=True, stop=True)
                    yt = y_pool.tile([128, QT], F32, tag="yt")
                    nc.scalar.activation(
                        yt[:], pt[:],
                        mybir.ActivationFunctionType.Identity,
                        bias=bo_sb[:, c : c + 1])
                    nc.sync.dma_start(y_d[c, :, s * QT : (s + 1) * QT],
                                      yt[:])


def _exp_schedule(n_units, n_act):
    """Bresenham-interleaved engine assignment: n_act of n_units to ACT."""
    sched = []
    acc = 0
    for _ in range(n_units):
        acc += n_act
        if acc >= n_units:
            acc -= n_units
            sched.append("act")
        else:
            sched.append("dve")
    return sched


N_ACT_KTILES_OF_32 = 18  # per-ktile engine split (ACT : DVE = 18 : 14)


def self_attention(nc, cfg, tc, exp_op, sim_ps, acc_ps, e_pool, norm_pool,
                   qT_sb, kT_sb, qT32_sb, kT32_sb, v_sb, outn_sb,
                   act_bias_sb, act_bias_nb_sb):
    NK, NQ, QT = cfg.n_ktiles, cfg.n_qtiles, cfg.q_tile
    sched = _exp_schedule(NK, N_ACT_KTILES_OF_32)

    pending_epilogue = [None]

    for qi in range(NQ):
        qs = slice(qi * QT, (qi + 1) * QT)
        accb = [acc_ps.tile([128, QT], F32, tag=f"acc{j}",
                            name=f"acc{j}_{qi}")
                for j in range(HEADS // 2)]

        def acc_sl(h, lo, hi):
            return accb[h // 2][64 * (h % 2) + lo : 64 * (h % 2) + hi, :]

        pend = []

        def emit_accs(et, pair, e_sb):
            for j in range(2):
                h = pair * 2 + j
                nc.tensor.matmul(
                    acc_sl(h, 0, 33),
                    v_sb[:, et, h, :],
                    e_sb[:, j, :],
                    start=(et == 0), stop=(et == NK - 1),
                    skip_group_check=True,
                )

        for t in range(NK):
            ts = slice(t * 128, (t + 1) * 128)
            eng = sched[t]
            sims = [sim_ps.tile([128, 2, QT], F32, tag="sim",
                                name=f"sim{qi}_{t}_{p}") for p in range(2)]
            if eng == "act":
                # 32-row sims (no bias row): all 4 heads pack into PE
                # row-tiles {0, 32, 64, 96} and run concurrently.
                for p in range(2):
                    for j in range(2):
                        h = 2 * p + j
                        nc.tensor.matmul(
                            sims[p][:, j, :],
                            kT32_sb[32 * h : 32 * h + 32, ts],
                            qT32_sb[32 * h : 32 * h + 32, qs],
                            start=True, stop=True,
                            tile_position=(32 * h, 0),
                        )
            else:
                # 33-row sims (bias row included): pairs pack 2-way into
                # row-tiles {0, 64}.
                for p in range(2):
                    for j in range(2):
                        nc.tensor.matmul(
                            sims[p][:, j, :],
                            kT_sb[64 * j : 64 * j + 33, p, ts],
                            qT_sb[64 * j : 64 * j + 33, p, qs],
                            start=True, stop=True,
                            tile_position=(64 * j, 0),
                        )
            for p in range(2):
                e_sb = e_pool.tile([128, 2, QT], BF16, tag="e",
                                   name=f"e{qi}_{t}_{p}")
                if eng == "act":
                    nc.scalar.activation(
                        e_sb[:], sims[p][:],
                        mybir.ActivationFunctionType.Exp,
                        bias=act_bias_nb_sb[:], scale=ACT_SCALE)
                else:
                    nc.vector._custom_dve(
                        exp_op,
                        out=e_sb[:].bitcast(I16),
                        in0=sims[p][:],
                        s0=CBIG, s1=A_FIT, imm2=C2_FIT)
                if DEFER_ACCS:
                    pend.append((t, p, e_sb))
                else:
                    emit_accs(t, p, e_sb)
            while len(pend) > 2:
                emit_accs(*pend.pop(0))
            if t == 1 and pending_epilogue[0] is not None:
                pending_epilogue[0]()
                pending_epilogue[0] = None
        while pend:
            emit_accs(*pend.pop(0))

        # ---- epilogue: normalize off-critical-path. Deferred into the
        # next q-tile (acc banks ping-pong via bufs=2) so its engine work
        # never delays the next q-tile's exp queues at the boundary.
        # Per-head base-0 scratch tiles: two-SBUF-input ops require equal
        # base partitions, and custom-DVE ops require aligned unstrided
        # APs; PSUM reads are exempt, so both scratch and denominator
        # copies read the acc banks directly.
        def make_epilogue(qi, qs, accb):
            def epilogue():
                scrs = [norm_pool.tile([32, QT], F32, tag=f"scr{h}",
                                       name=f"scr{h}_{qi}")
                        for h in range(HEADS)]
                r4 = norm_pool.tile([1, HEADS, QT], F32, tag="r4",
                                    name=f"r4_{qi}")
                for h in range(HEADS):
                    nc.scalar.copy(
                        scrs[h][:],
                        accb[h // 2][64 * (h % 2) : 64 * (h % 2) + 32, :])
                    nc.vector.tensor_copy(
                        r4[0:1, h, :],
                        accb[h // 2][32 + 64 * (h % 2) : 33 + 64 * (h % 2),
                                     :])
                for h in range(HEADS):
                    nc.vector.reciprocal_approx_fast(
                        r4[0:1, h, :], r4[0:1, h, :])
                bcs = [norm_pool.tile([32, QT], F32, tag=f"bc{h}",
                                      name=f"bc{h}_{qi}")
                       for h in range(HEADS)]
                for h in range(HEADS):
                    nc.gpsimd.partition_broadcast(bcs[h][:], r4[0:1, h, :])
                for h in range(HEADS):
                    nc.vector.tensor_mul(
                        outn_sb[32 * h : 32 * h + 32, qs],
                        scrs[h][:], bcs[h][:])
            return epilogue

        if qi < NQ - 1:
            pending_epilogue[0] = make_epilogue(qi, qs, accb)
        else:
            make_epilogue(qi, qs, accb)()


# ---------------------------------------------------------------------
# host side
# ---------------------------------------------------------------------

def make_in_maps(x, w_qkv, w_out, b_out, cfg=FULL, n_cores=N_CORES):
    b, dim, H, W = x.shape
    seq = H * W
    bf = ml_dtypes.bfloat16

    wq = (w_qkv[0:128] * (SCALE * K1)).astype(np.float32)
    wk = w_qkv[128:256]
    wv = w_qkv[256:384]
    wq_t = np.ascontiguousarray(
        wq.T.reshape(cfg.n_ctiles, 128, 128)).astype(bf)
    wk_t = np.ascontiguousarray(
        wk.T.reshape(cfg.n_ctiles, 128, 128)).astype(bf)
    wv_t = np.ascontiguousarray(
        wv.T.reshape(cfg.n_ctiles, 128, 128)).astype(bf)
    wo_t = np.ascontiguousarray(w_out.T).astype(bf)
    bo = b_out.reshape(cfg.n_ctiles, 128, 1).astype(np.float32)

    in_maps = []
    for core in range(n_cores):
        bi, half = core // 2, core % 2
        xb = x[bi].reshape(dim, seq)
        x_bf = xb.reshape(cfg.n_ctiles, 128, seq).astype(bf)
        xq_bf = np.ascontiguousarray(
            xb[:, half * cfg.q_half : (half + 1) * cfg.q_half]
        ).reshape(cfg.n_ctiles, 128, cfg.q_half).astype(bf)
        in_maps.append({
            "x": x_bf, "xq": xq_bf,
            "wq": wq_t, "wk": wk_t, "wv": wv_t,
            "wo": wo_t, "bo": bo,
        })
    return in_maps


def assemble_output(results, x_shape, cfg=FULL):
    b, dim, H, W = x_shape
    out = np.empty((b, dim, H * W), dtype=np.float32)
    for core, r in enumerate(results):
        bi, half = core // 2, core % 2
        y = r["out"].reshape(dim, cfg.q_half)
        out[bi, :, half * cfg.q_half : (half + 1) * cfg.q_half] = y
    return out.reshape(b, dim, H, W)


_CACHE = {}


def _get_nc():
    if "nc" not in _CACHE:
        _CACHE["nc"] = build_nc()
    return _CACHE["nc"]


def kernel(x, w_qkv, w_out, b_out, trace=False):
    from concourse.bass_utils import run_bass_kernel_spmd

    nc = _get_nc()
    in_maps = make_in_maps(np.asarray(x), np.asarray(w_qkv),
                           np.asarray(w_out), np.asarray(b_out))
    last_err = None
    for _attempt in range(4):
        try:
            res = run_bass_kernel_spmd(nc, in_maps,
                                       core_ids=list(range(N_CORES)),
                                       trace=trace)
            break
        except Exception as e:  # transient NRT device errors
            last_err = e
            res = None
    if res is None:
        raise last_err
    _CACHE["last_result"] = res
    return assemble_output(res.results, np.asarray(x).shape)


# revision 13
# speedup vs baseline: 1.2207x; 1.2207x over previous
"""Distributed Trainium2 Bass kernel for the 1x1-conv multi-head attention block.

Reference computation (per batch b of 4):
    qkv = w_qkv @ x            x: [256, 4096] (channels x spatial), w_qkv: [384, 256]
    q,k,v = split(qkv); per head h (4 heads, d=32): q *= d**-0.5
    sim = q^T k;  attn = softmax(sim, axis=k);  o = attn @ v^T
    y = w_out @ o + b_out      w_out: [256, 128]

Distribution: 8 cores = 4 batches x 2 query-halves. Each core computes k/v
projections for all 4096 positions, q for its 2048 query positions, full
attention for its query half, and the output projection for those columns.
The host concatenates the 8 disjoint output blocks; no collective needed.

The kernel is exp-throughput-bound (33.5M softmax exponentials per core, ACT
does 128 lanes/cycle), so the exp work is split between two engines:
  - ACT: native Exp activation (PSUM fp32 -> SBUF bf16).
  - DVE: a custom 8-stage DVE op (EXP_BITS, installed over the unused
    ADD_RANGE_WRAP table row) that emits bf16 BIT PATTERNS via int16
    convert: t = rho + Fm + c2*(Fm^2 + A), where rho/Fm are the 128-aligned
    round/remainder of the pre-scaled score (magic-constant rounding) and
    the quadratic corrects the mantissa field. Max value error ~0.5%.
Scores are computed pre-scaled by K1 = 128*log2(e) (folded into w_q) and
pre-biased by B = 16192 (a 33rd contraction row in the q/k layouts) so the
custom op's frac extraction is wrap-free. The ACT path compensates with its
free scale/bias to produce the same global factor.

Layouts: scores simT [k_pos (partitions), q_pos (free)]; softmax denominators
via a ones-column in the v stationary (PE partition reduction); attn@v
consumes exp'd scores as the bf16 moving operand. q/k live in a pitch-64
layout (head pair per tile: rows 0-32 and 64-96 incl. the aug row) so the
33-row sim matmuls 2-way-pack into PE row-tile positions {0, 64}.
"""

import sys

if "/opt/trn_rl_repo" not in sys.path:
    sys.path.insert(0, "/opt/trn_rl_repo")

import numpy as np
import ml_dtypes

import concourse.bass as bass
import concourse.mybir as mybir
import concourse.tile as tile
from concourse import bacc
from concourse import dve_ops
from concourse.dve_spec import Spec, Src0, C0, C1, C2, lower as dve_lower
from concourse.dve_uop import DveOpSpec

BF16 = mybir.dt.bfloat16
F32 = mybir.dt.float32
I16 = mybir.dt.int16

N_CORES = 8
HEADS = 4
DIM_HEAD = 32
SCALE = DIM_HEAD ** -0.5

# ---- exp constants (see probe_dve.py derivation) ------------------------
K1 = float(128.0 * np.log2(np.e))        # score pre-scale (folded into wq)
BIAS_B = 16192.0                         # score pre-bias (aug row), == 64 mod 128
CBIG = float(np.float32(1.5 * 2 ** 30))  # magic round-to-128 constant
C2_FIT = 0.0026719477027654648           # mantissa parabola curvature
A_FIT = 19838.392578125                  # parabola constant / c2
GAMMA = 0.999614                         # global factor of the DVE path
ACT_SCALE = float(1.0 / K1)
ACT_BIAS = float(np.log(GAMMA) - BIAS_B / K1)

# exp engine split: of every 64 (ktile, pair) units per q-tile, this many
# go to ACT (rest to the DVE custom op)
N_ACT_OF_64 = 35

DEFER_ACCS = True   # emit attn@v matmuls one k-tile late (engine continuity)


# ---- custom DVE op installation -----------------------------------------
# New table rows crash the runtime in this environment, so EXP_BITS is
# installed over an existing row (ADD_RANGE_WRAP, unused here). The per-NEFF
# DVE table is regenerated from dve_ops.OPS at compile time.

_EXP_OP = None


def _install_exp_op():
    global _EXP_OP
    name = "ADD_RANGE_WRAP"
    if _EXP_OP is not None and dve_ops.CUSTOM_DVE_SPECS.get(name) is _EXP_OP.spec:
        return _EXP_OP
    a = Src0 + C0
    rho = a - C0
    fm = Src0 - rho
    v = fm * fm
    u = v + C1
    psi = u * C2
    g = psi + fm
    body = g + rho

    def ref(in0, in1, s0, s1, imm2):
        z = in0.astype(np.float32)
        aa = (z + np.float32(s0)).astype(np.float32)
        rr = (aa - np.float32(s0)).astype(np.float32)
        ff = (z - rr).astype(np.float32)
        vv = (ff * ff).astype(np.float32)
        uu = (vv + np.float32(s1)).astype(np.float32)
        pp = (uu * np.float32(imm2)).astype(np.float32)
        gg = (pp + ff).astype(np.float32)
        return (gg + rr).astype(np.float32)

    op = dve_ops.DveOp(name, Spec(body=body, reference=ref),
                       subdim=False, uops_sha={})
    row = dve_ops._SUB_OPCODE_FOR_NAME[name]
    for ver in ("v3", "v4"):
        sp = DveOpSpec(name=name, opcode=row,
                       uops=dve_lower(op.spec, ver=ver), rd1_en=False)
        op.uops_sha[ver] = sp.sha(ver)
    idx = [i for i, o in enumerate(dve_ops.OPS) if o.name == name][0]
    dve_ops.OPS[idx] = op
    dve_ops.CUSTOM_DVE_SPECS[name] = op.spec
    _EXP_OP = op
    return op


class Cfg:
    def __init__(self, seq=4096, q_half=2048, q_tile=512, dim=256):
        self.seq = seq
        self.q_half = q_half
        self.q_tile = q_tile
        self.dim = dim
        self.hidden = HEADS * DIM_HEAD
        self.n_ktiles = seq // 128
        self.n_qtiles = q_half // q_tile
        self.n_stiles = seq // q_tile
        self.n_ctiles = dim // 128


FULL = Cfg()


def build_nc(cfg=FULL, n_cores=N_CORES):
    exp_op = _install_exp_op()
    nc = bacc.Bacc("TRN2", target_bir_lowering=False, debug=False,
                   num_devices=n_cores)

    x_d = nc.dram_tensor("x", [cfg.n_ctiles, 128, cfg.seq], BF16,
                         kind="ExternalInput")
    xq_d = nc.dram_tensor("xq", [cfg.n_ctiles, 128, cfg.q_half], BF16,
                          kind="ExternalInput")
    wq_d = nc.dram_tensor("wq", [cfg.n_ctiles, 128, 128], BF16,
                          kind="ExternalInput")
    wk_d = nc.dram_tensor("wk", [cfg.n_ctiles, 128, 128], BF16,
                          kind="ExternalInput")
    wv_d = nc.dram_tensor("wv", [cfg.n_ctiles, 128, 128], BF16,
                          kind="ExternalInput")
    wo_d = nc.dram_tensor("wo", [128, cfg.dim], BF16, kind="ExternalInput")
    bo_d = nc.dram_tensor("bo", [cfg.n_ctiles, 128, 1], F32,
                          kind="ExternalInput")
    y_d = nc.dram_tensor("out", [cfg.n_ctiles, 128, cfg.q_half], F32,
                         kind="ExternalOutput")

    with tile.TileContext(nc) as tc:
        _kernel_body(tc, cfg, exp_op, x_d, xq_d, wq_d, wk_d, wv_d, wo_d,
                     bo_d, y_d)
    nc.compile()
    return nc


def _kernel_body(tc, cfg, exp_op, x_d, xq_d, wq_d, wk_d, wv_d, wo_d, bo_d,
                 y_d):
    nc = tc.nc
    NK, NQ, QT = cfg.n_ktiles, cfg.n_qtiles, cfg.q_tile
    NC_, NS = cfg.n_ctiles, cfg.n_stiles

    from contextlib import ExitStack
    with ExitStack() as ctx:
        ep = ctx.enter_context

        consts = ep(tc.tile_pool(name="consts", bufs=1))
        persist = ep(tc.tile_pool(name="persist", bufs=1))

        x_sb = persist.tile([128, NC_, cfg.seq], BF16, tag="x")
        xq_sb = persist.tile([128, NC_, cfg.q_half], BF16, tag="xq")
        wq_sb = consts.tile([128, NC_, 128], BF16, tag="wq")
        wk_sb = consts.tile([128, NC_, 128], BF16, tag="wk")
        wv_sb = consts.tile([128, NC_, 128], BF16, tag="wv")
        wo_sb = consts.tile([128, cfg.dim], BF16, tag="wo")
        bo_sb = consts.tile([128, NC_], F32, tag="bo")

        # Prime the ACT exp table before the PE-dense phases (one-time
        # ~1.3us table load otherwise lands on the first real exp).
        prime = consts.tile([128, 8], F32, tag="prime")
        nc.vector.memset(prime[:], 0.0)
        nc.scalar.activation(prime[:], prime[:],
                             mybir.ActivationFunctionType.Exp)

        act_bias_sb = consts.tile([128, 1], F32, tag="act_bias")
        nc.vector.memset(act_bias_sb[:], ACT_BIAS)
        act_bias_nb_sb = consts.tile([128, 1], F32, tag="act_bias_nb")
        nc.vector.memset(act_bias_nb_sb[:], float(np.log(GAMMA)))

        for c in range(NC_):
            nc.sync.dma_start(xq_sb[:, c, :], xq_d[c])
            nc.sync.dma_start(wq_sb[:, c, :], wq_d[c])
        for c in range(NC_):
            nc.sync.dma_start(wk_sb[:, c, :], wk_d[c])
            nc.sync.dma_start(wv_sb[:, c, :], wv_d[c])
            nc.sync.dma_start(x_sb[:, c, :], x_d[c])
        for c in range(NC_):
            nc.sync.dma_start(bo_sb[:, c : c + 1], bo_d[c])
        nc.sync.dma_start(wo_sb[:], wo_d[:])

        # ---- projections ------------------------------------------------
        # Packed layouts (head h at rows 32h) feed the 4-way ACT-ktile
        # sims; pitch-64 aug layouts (head pair p: rows 0-32 / 64-96 incl.
        # the bias row) feed the 33-row DVE-ktile sims. The pitch-64
        # copies are SBUF->SBUF DMAs from the packed layout (DMA engines
        # are idle; engines can't cross partitions cheaply).
        qT32_sb = persist.tile([128, cfg.q_half], BF16, tag="qT32")
        kT32_sb = persist.tile([128, cfg.seq], BF16, tag="kT32")
        qT_sb = persist.tile([128, 2, cfg.q_half], BF16, tag="qT")
        kT_sb = persist.tile([128, 2, cfg.seq], BF16, tag="kT")
        v_sb = persist.tile([128, NK, HEADS, 33], BF16, tag="v")

        # aug rows (partition rows 32 / 96 of each pair slot)
        nc.vector.memset(qT_sb[32:33, :, :], BIAS_B)
        nc.vector.memset(qT_sb[96:97, :, :], BIAS_B)
        nc.vector.memset(kT_sb[32:33, :, :], 1.0)
        nc.vector.memset(kT_sb[96:97, :, :], 1.0)
        nc.vector.memset(v_sb[:, :, :, 32], 1.0)

        with tc.tile_pool(name="proj_ps", bufs=4, space="PSUM") as proj_ps, \
             tc.tile_pool(name="projv_ps", bufs=4, space="PSUM") as projv_ps:
            for s in range(cfg.q_half // QT):
                pt = proj_ps.tile([128, QT], F32, tag="proj")
                for c in range(NC_):
                    nc.tensor.matmul(pt[:], wq_sb[:, c, :],
                                     xq_sb[:, c, s * QT : (s + 1) * QT],
                                     start=(c == 0), stop=(c == NC_ - 1))
                nc.vector.tensor_copy(
                    qT32_sb[:, s * QT : (s + 1) * QT], pt[:])
            for h in range(HEADS):
                nc.sync.dma_start(
                    qT_sb[64 * (h % 2) : 64 * (h % 2) + 32, h // 2, :],
                    qT32_sb[32 * h : 32 * h + 32, :])
            for s in range(NS):
                pt = proj_ps.tile([128, QT], F32, tag="proj")
                for c in range(NC_):
                    nc.tensor.matmul(pt[:], wk_sb[:, c, :],
                                     x_sb[:, c, s * QT : (s + 1) * QT],
                                     start=(c == 0), stop=(c == NC_ - 1))
                eng = nc.scalar.copy if s % 2 == 0 else nc.vector.tensor_copy
                eng(kT32_sb[:, s * QT : (s + 1) * QT], pt[:])
            for h in range(HEADS):
                nc.sync.dma_start(
                    kT_sb[64 * (h % 2) : 64 * (h % 2) + 32, h // 2, :],
                    kT32_sb[32 * h : 32 * h + 32, :])
            # v projection, spatial-major: out [128 spatial, 128 hidden]
            for t in range(NK):
                pt = projv_ps.tile([128, 128], F32, tag="projv")
                for c in range(NC_):
                    nc.tensor.matmul(pt[:],
                                     x_sb[:, c, t * 128 : (t + 1) * 128],
                                     wv_sb[:, c, :],
                                     start=(c == 0), stop=(c == NC_ - 1))
                nc.scalar.copy(
                    v_sb[:, t, :, 0:32],
                    pt[:].rearrange("p (h d) -> p h d", h=HEADS))

        # ---- attention --------------------------------------------------
        attn_ctx = ExitStack()
        with attn_ctx:
            sim_ps = attn_ctx.enter_context(
                tc.tile_pool(name="sim_ps", bufs=3, space="PSUM"))
            acc_ps = attn_ctx.enter_context(
                tc.tile_pool(name="acc_ps", bufs=1, space="PSUM"))
            e_pool = attn_ctx.enter_context(
                tc.tile_pool(name="e_pool", bufs=3))
            norm_pool = attn_ctx.enter_context(
                tc.tile_pool(name="norm", bufs=3))

            outn_sb = persist.tile([128, cfg.q_half], BF16, tag="outn")

            self_attention(nc, cfg, tc, exp_op, sim_ps, acc_ps, e_pool,
                           norm_pool, qT_sb, kT_sb, qT32_sb, kT32_sb,
                           v_sb, outn_sb, act_bias_sb, act_bias_nb_sb)

        # ---- output projection ------------------------------------------
        with tc.tile_pool(name="y", bufs=8) as y_pool, \
             tc.tile_pool(name="y_ps", bufs=8, space="PSUM") as yp_ps:
            for c in range(NC_):
                for s in range(cfg.q_half // QT):
                    pt = yp_ps.tile([128, QT], F32, tag="yp")
                    nc.tensor.matmul(pt[:],
                                     wo_sb[:, c * 128 : (c + 1) * 128],
                                     outn_sb[:, s * QT : (s + 1) * QT],
                                     start=True, stop=True)
                    yt = y_pool.tile([128, QT], F32, tag="yt")
                    nc.scalar.activation(
                        yt[:], pt[:],
                        mybir.ActivationFunctionType.Identity,
                        bias=bo_sb[:, c : c + 1])
                    nc.sync.dma_start(y_d[c, :, s * QT : (s + 1) * QT],
                                      yt[:])


def _exp_schedule(n_units, n_act):
    """Bresenham-interleaved engine assignment: n_act of n_units to ACT."""
    sched = []
    acc = 0
    for _ in range(n_units):
        acc += n_act
        if acc >= n_units:
            acc -= n_units
            sched.append("act")
        else:
            sched.append("dve")
    return sched


N_ACT_KTILES_OF_32 = 18  # per-ktile engine split (ACT : DVE = 18 : 14)


def self_attention(nc, cfg, tc, exp_op, sim_ps, acc_ps, e_pool, norm_pool,
                   qT_sb, kT_sb, qT32_sb, kT32_sb, v_sb, outn_sb,
                   act_bias_sb, act_bias_nb_sb):
    NK, NQ, QT = cfg.n_ktiles, cfg.n_qtiles, cfg.q_tile
    sched = _exp_schedule(NK, N_ACT_KTILES_OF_32)

    pending_epilogue = [None]

    for qi in range(NQ):
        qs = slice(qi * QT, (qi + 1) * QT)
        accb = [acc_ps.tile([128, QT], F32, tag=f"acc{j}",
                            name=f"acc{j}_{qi}")
                for j in range(HEADS // 2)]

        def acc_sl(h, lo, hi):
            return accb[h // 2][64 * (h % 2) + lo : 64 * (h % 2) + hi, :]

        pend = []

        def emit_accs(et, pair, e_sb):
            for j in range(2):
                h = pair * 2 + j
                nc.tensor.matmul(
                    acc_sl(h, 0, 33),
                    v_sb[:, et, h, :],
                    e_sb[:, j, :],
                    start=(et == 0), stop=(et == NK - 1),
                    skip_group_check=True,
                )

        for t in range(NK):
            ts = slice(t * 128, (t + 1) * 128)
            eng = sched[t]
            sims = [sim_ps.tile([128, 2, QT], F32, tag="sim",
                                name=f"sim{qi}_{t}_{p}") for p in range(2)]
            if eng == "act":
                # 32-row sims (no bias row): all 4 heads pack into PE
                # row-tiles {0, 32, 64, 96} and run concurrently.
                for p in range(2):
                    for j in range(2):
                        h = 2 * p + j
                        nc.tensor.matmul(
                            sims[p][:, j, :],
                            kT32_sb[32 * h : 32 * h + 32, ts],
                            qT32_sb[32 * h : 32 * h + 32, qs],
                            start=True, stop=True,
                            tile_position=(32 * h, 0),
                        )
            else:
                # 33-row sims (bias row included): pairs pack 2-way into
                # row-tiles {0, 64}.
                for p in range(2):
                    for j in range(2):
                        nc.tensor.matmul(
                            sims[p][:, j, :],
                            kT_sb[64 * j : 64 * j + 33, p, ts],
                            qT_sb[64 * j : 64 * j + 33, p, qs],
                            start=True, stop=True,
                            tile_position=(64 * j, 0),
                        )
            for p in range(2):
                e_sb = e_pool.tile([128, 2, QT], BF16, tag="e",
                                   name=f"e{qi}_{t}_{p}")
                if eng == "act":
                    nc.scalar.activation(
                        e_sb[:], sims[p][:],
                        mybir.ActivationFunctionType.Exp,
                        bias=act_bias_nb_sb[:], scale=ACT_SCALE)
                else:
                    nc.vector._custom_dve(
                        exp_op,
                        out=e_sb[:].bitcast(I16),
                        in0=sims[p][:],
                        s0=CBIG, s1=A_FIT, imm2=C2_FIT)
                if DEFER_ACCS:
                    pend.append((t, p, e_sb))
                else:
                    emit_accs(t, p, e_sb)
            while len(pend) > 2:
                emit_accs(*pend.pop(0))
        while pend:
            emit_accs(*pend.pop(0))

        # ---- epilogue: normalize off-critical-path. Deferred into the
        # next q-tile (acc banks ping-pong via bufs=2) so its engine work
        # never delays the next q-tile's exp queues at the boundary.
        # Per-head base-0 scratch tiles: two-SBUF-input ops require equal
        # base partitions, and custom-DVE ops require aligned unstrided
        # APs; PSUM reads are exempt, so both scratch and denominator
        # copies read the acc banks directly.
        # Free the two acc banks with one full-bank copy each, issued on
        # DIFFERENT engines so they run concurrently (~0.6us): the k-tile
        # boundary PE stall is what drops the PE clock to half speed.
        # Everything downstream reads the SBUF scratch, not the banks.
        def make_epilogue(qi, qs, accb):
            def epilogue():
                scr = norm_pool.tile([128, 2, QT], F32, tag="scr",
                                     name=f"scr_{qi}")
                nc.scalar.copy(scr[:, 0, :], accb[0][:])
                nc.vector.tensor_copy(scr[:, 1, :], accb[1][:])
                r4 = norm_pool.tile([1, HEADS, QT], F32, tag="r4",
                                    name=f"r4_{qi}")
                for h in range(HEADS):
                    nc.scalar.copy(
                        r4[0:1, h, :],
                        scr[32 + 64 * (h % 2) : 33 + 64 * (h % 2),
                            h // 2, :])
                for h in range(HEADS):
                    nc.vector.reciprocal_approx_fast(
                        r4[0:1, h, :], r4[0:1, h, :])
                # broadcast writes partitions 0..31 regardless of the dst
                # AP base, so bc tiles are base-0; odd heads' scratch rows
                # (base 64) get a base-0 shift copy to satisfy the
                # two-SBUF-input equal-base rule on the muls.
                scr0 = [None] * HEADS
                for h in range(HEADS):
                    if h % 2 == 0:
                        scr0[h] = scr[0:32, h // 2, :]
                    else:
                        t0 = norm_pool.tile([32, QT], F32, tag=f"s0{h}",
                                            name=f"s0{h}_{qi}")
                        eng = nc.scalar.copy if h == 1 \
                            else nc.vector.tensor_copy
                        eng(t0[:], scr[64:96, h // 2, :])
                        scr0[h] = t0[:]
                bcs = [norm_pool.tile([32, QT], F32, tag=f"bc{h}",
                                      name=f"bc{h}_{qi}")
                       for h in range(HEADS)]
                for h in range(HEADS):
                    nc.gpsimd.partition_broadcast(bcs[h][:], r4[0:1, h, :])
                for h in range(HEADS):
                    nc.vector.tensor_mul(
                        outn_sb[32 * h : 32 * h + 32, qs],
                        scr0[h], bcs[h][:])
            return epilogue

        make_epilogue(qi, qs, accb)()


# ---------------------------------------------------------------------
# host side
# ---------------------------------------------------------------------

def make_in_maps(x, w_qkv, w_out, b_out, cfg=FULL, n_cores=N_CORES):
    b, dim, H, W = x.shape
    seq = H * W
    bf = ml_dtypes.bfloat16

    wq = (w_qkv[0:128] * (SCALE * K1)).astype(np.float32)
    wk = w_qkv[128:256]
    wv = w_qkv[256:384]
    wq_t = np.ascontiguousarray(
        wq.T.reshape(cfg.n_ctiles, 128, 128)).astype(bf)
    wk_t = np.ascontiguousarray(
        wk.T.reshape(cfg.n_ctiles, 128, 128)).astype(bf)
    wv_t = np.ascontiguousarray(
        wv.T.reshape(cfg.n_ctiles, 128, 128)).astype(bf)
    wo_t = np.ascontiguousarray(w_out.T).astype(bf)
    bo = b_out.reshape(cfg.n_ctiles, 128, 1).astype(np.float32)

    in_maps = []
    for core in range(n_cores):
        bi, half = core // 2, core % 2
        xb = x[bi].reshape(dim, seq)
        x_bf = xb.reshape(cfg.n_ctiles, 128, seq).astype(bf)
        xq_bf = np.ascontiguousarray(
            xb[:, half * cfg.q_half : (half + 1) * cfg.q_half]
        ).reshape(cfg.n_ctiles, 128, cfg.q_half).astype(bf)
        in_maps.append({
            "x": x_bf, "xq": xq_bf,
            "wq": wq_t, "wk": wk_t, "wv": wv_t,
            "wo": wo_t, "bo": bo,
        })
    return in_maps


def assemble_output(results, x_shape, cfg=FULL):
    b, dim, H, W = x_shape
    out = np.empty((b, dim, H * W), dtype=np.float32)
    for core, r in enumerate(results):
        bi, half = core // 2, core % 2
        y = r["out"].reshape(dim, cfg.q_half)
        out[bi, :, half * cfg.q_half : (half + 1) * cfg.q_half] = y
    return out.reshape(b, dim, H, W)


_CACHE = {}


def _get_nc():
    if "nc" not in _CACHE:
        _CACHE["nc"] = build_nc()
    return _CACHE["nc"]


def kernel(x, w_qkv, w_out, b_out, trace=False):
    from concourse.bass_utils import run_bass_kernel_spmd

    nc = _get_nc()
    in_maps = make_in_maps(np.asarray(x), np.asarray(w_qkv),
                           np.asarray(w_out), np.asarray(b_out))
    last_err = None
    for _attempt in range(4):
        try:
            res = run_bass_kernel_spmd(nc, in_maps,
                                       core_ids=list(range(N_CORES)),
                                       trace=trace)
            break
        except Exception as e:  # transient NRT device errors
            last_err = e
            res = None
    if res is None:
        raise last_err
    _CACHE["last_result"] = res
    return assemble_output(res.results, np.asarray(x).shape)
